# revision 21
# baseline (speedup 1.0000x reference)
"""Bloom self-attention wrapper kernel for Trainium2 (8 NeuronCores).

Shards the 16 attention heads across 8 cores (2 heads/core, tensor
parallel).  Each core computes its heads' q/k/v projection, attention
probabilities p, the per-dim softmax deviations ("qk importance"), the
attention context and a partial dense output.  The host sums the dense
partials (the all-reduce), adds bias + residual, and concatenates the
per-head importance slices.

Key device-side structure per (head, d):  the exponent tile
x_d[i,j] = (qk[i,j] - q[i,d]k[j,d])/64 + mask[i,j]/8 is built entirely
on the tensor engine in PSUM (full-qk matmul + rank-1 matmul + mask add
via identity matmul, all float32r), the scalar engine computes
T = exp(x) with a fused row-sum (Z), and a custom fused DVE op computes
sum_j (T*recipZ - p)^2 in a single vector-engine pass.
"""

import math
import operator
import os
import sys
from contextlib import ExitStack

import numpy as np

sys.path.insert(0, "/opt/trn_rl_repo")

import concourse.bacc as bacc  # noqa: E402
import concourse.bass as bass  # noqa: E402
import concourse.tile as tile  # noqa: E402
from concourse import mybir  # noqa: E402
from concourse._compat import with_exitstack  # noqa: E402

F32 = mybir.dt.float32
F32R = mybir.dt.float32r
F16 = mybir.dt.float16
AX = mybir.AxisListType
ACT = mybir.ActivationFunctionType

B, S, HID = 1, 256, 1024
H, D = 16, 64
NC_ = 8          # cores
HL = 2           # heads per core
KCH = 9          # contraction chunks for the (1152 = 9*128)-row padded projection
KPAD = KCH * 128


# ---------------------------------------------------------------------------
# Custom DVE op: out = (in0 - s0*in1)^2 ; accum_out = sum(out) per partition.
# With in0 = T, s0 = Z, in1 = p this gives sum_j (T - Z p)^2 = Z^2 sum (pd-p)^2
# so the reciprocal of Z moves out of the inner loop entirely.
# ---------------------------------------------------------------------------
def _register_sqdiff_op():
    from concourse import dve_ops
    from concourse.dve_spec import C0, Spec, Src0, Src1, Zero, lower, sq

    name = "SQDIFFZ_REDUCE_K"
    for op in dve_ops.OPS:
        if op.name == name:
            return op

    def _ref(in0, in1, c0, c1, c2):
        b = np.square(in0.astype(np.float32) - c0 * in1.astype(np.float32))
        b = b.astype(np.float32)
        return b, b.reshape(b.shape[0], -1).sum(axis=-1, keepdims=True)

    spec = Spec(
        body=sq(Src0 - C0 * Src1),
        accum=operator.add,
        accum_init=Zero,
        reference=_ref,
    )
    row = dve_ops._CUSTOM_DVE_ROW_BASE + len(dve_ops.OPS)
    assert row < 0x20, "custom DVE opcode row overflow"
    from concourse.dve_uop import DveOpSpec

    shas = {}
    for ver in ("v3", "v4"):
        try:
            tmp = DveOpSpec(
                name=name, opcode=row, uops=lower(spec, ver=ver), rd1_en=True
            )
            shas[ver] = tmp.sha(ver)
        except Exception:
            pass
    op = dve_ops.DveOp(name, spec, subdim=False, uops_sha=shas)
    dve_ops.OPS.append(op)
    dve_ops._SUB_OPCODE_FOR_NAME[name] = row
    dve_ops.CUSTOM_DVE_SPECS[name] = spec
    return op


SQDIFF_OP = _register_sqdiff_op()
NO_CUSTOM = bool(int(os.environ.get("KERNEL_NO_CUSTOM", "0")))


def _dup2(ap):
    """[P, N] access pattern -> [P, 2, N] with a step-0 middle dim."""
    assert len(ap.ap) == 2, ap.ap
    return bass.AP(
        tensor=ap.tensor, offset=ap.offset, ap=[ap.ap[0], [0, 2], ap.ap[1]]
    )


def _r(ap):
    return ap.bitcast(F32R)


# ---------------------------------------------------------------------------
# Device kernel (one core, SPMD: identical program, per-core data)
# ---------------------------------------------------------------------------
@with_exitstack
def _tile_kernel(ctx: ExitStack, tc: tile.TileContext, outs, ins):
    nc = tc.nc
    hid_d, what_d, wd_d, mask8_d, al8_d, i128_d, ones_d, zrow_d = ins
    outp_d, imp_d = outs

    const = ctx.enter_context(tc.tile_pool(name="const", bufs=1))
    small = ctx.enter_context(tc.tile_pool(name="small", bufs=4))

    # ---- input DMA ---------------------------------------------------------
    # Tiles feeding float32r matmuls are declared float32r (BIR verifier
    # requires f32r-matmul operands to be produced as f32r).
    hid_sb = const.tile([128, KCH, S], F32R)
    what_sb = const.tile([128, KCH, 3 * 128], F32R)
    hid_r = hid_d.rearrange("(c p) n -> p c n", p=128).bitcast(F32R)
    what_r = what_d.rearrange("(c p) n -> p c n", p=128).bitcast(F32R)
    for kc in range(KCH):
        nc.gpsimd.dma_start(what_sb[:, kc, :], what_r[:, kc, :])
        nc.gpsimd.dma_start(hid_sb[:, kc, :], hid_r[:, kc, :])
    wd_h = []
    for hh in range(HL):
        w = const.tile([64, 1024], F32R, name=f"wd_h{hh}")
        nc.sync.dma_start(w, wd_d[hh * 64 : (hh + 1) * 64].bitcast(F32R))
        wd_h.append(w)
    m8_i0 = const.tile([128, S], F32R)
    nc.sync.dma_start(m8_i0, mask8_d[0:128].bitcast(F32R))
    m8_i1 = const.tile([128, S], F32R)
    nc.sync.dma_start(m8_i1, mask8_d[128:256].bitcast(F32R))
    i128r_sb = const.tile([128, 128], F32R)
    nc.sync.dma_start(i128r_sb, i128_d.bitcast(F32R))
    i128f_sb = const.tile([128, 128], F32)
    nc.sync.dma_start(i128f_sb, i128_d)
    al8_h = []
    for hh in range(HL):
        a = const.tile([1, S], F32R, name=f"al8_h{hh}")
        nc.sync.dma_start(a, al8_d[hh : hh + 1].bitcast(F32R))
        al8_h.append(a)

    ones_row = const.tile([1, S], F32R)
    nc.sync.dma_start(ones_row, ones_d.bitcast(F32R))

    dramp = ctx.enter_context(tc.tile_pool(name="dramp", bufs=1, space="DRAM"))
    kdimg = dramp.tile([128, D * S], F16)
    NCH = 4
    CW = D // NCH
    for ch in range(NCH):
        zbc = bass.AP(tensor=zrow_d.tensor, offset=zrow_d.offset,
                      ap=[[0, 128], [1, CW * S]])
        nc.sync.dma_start(kdimg[:, ch * CW * S : (ch + 1) * CW * S], zbc)
    ones_col = const.tile([128, 1], F32)
    nc.vector.memset(ones_col, 1.0)

    # kdiag: row r = (h d'), free [d, j]; kdiag[r, d, :] = k_s[r, :] iff
    # d == r mod 64 else 0.  Lets the per-d rank-1 subtraction run as an
    # aligned K=64 matmul (PE requires quadrant-aligned partition bases).
    # Allocated + zeroed up front so its lifetime overlaps every const tile.
    kdiag = const.tile([128, D, S], F16)

    # ---- projection: q_s, k_s (scaled 1/8, bias folded), v ----------------
    q_s = const.tile([128, S], F32R)     # [(h d), i]
    k_s = const.tile([128, S], F32R)     # [(h d), j]
    k_f16 = const.tile([128, S], F16)
    qneg = const.tile([128, S], F16)
    v_dm = const.tile([128, S], F32)    # [(h d), j]
    v_im = [const.tile([128, 128], F32R, name=f"v_im{i}") for i in range(2)]  # [j, (h d)] blocks

    with tc.tile_pool(name="proj_ps", bufs=2, space="PSUM") as pp, tc.tile_pool(
        name="tr_ps", bufs=2, space="PSUM"
    ) as tp:
        for idx, dst in enumerate((q_s, k_s, v_dm)):
            ps = pp.tile([128, S], F32)
            cb = idx * 128
            for kc in range(KCH):
                nc.tensor.matmul(
                    ps,
                    what_sb[:, kc, cb : cb + 128],
                    hid_sb[:, kc, :],
                    start=(kc == 0),
                    stop=(kc == KCH - 1),
                )
            nc.scalar.copy(dst, ps)
            if idx == 1:
                nc.scalar.copy(k_f16, ps)
        for ib in range(2):
            tps = tp.tile([128, 128], F32)
            nc.tensor.transpose(tps, v_dm[:, ib * 128 : (ib + 1) * 128], i128f_sb)
            nc.scalar.copy(v_im[ib], tps)
    nc.scalar.mul(qneg, q_s.bitcast(F32), -1.0)

    # kdiag built via a DRAM image, pipelined in NCH chunks of CW diagonal
    # positions: zero-fill (issued up top), strided scatter (the diagonal
    # destination offset r*16384 + (r%64)*256 is linear in r within each
    # 64-row half), then readback into SBUF.
    for ch in range(NCH):
        r0 = ch * CW
        for half in range(2):
            dst = bass.AP(
                tensor=kdimg.tensor,
                offset=kdimg.offset + half * 64 * (D * S) + r0 * (D * S + S),
                ap=[[D * S + S, CW], [1, S]],
            )
            nc.sync.dma_start(
                dst, k_f16[half * 64 + r0 : half * 64 + r0 + CW, :]
            )
        nc.sync.dma_start(
            kdiag[:, r0 : r0 + CW, :],
            kdimg[:, r0 * S : (r0 + CW) * S],
        )

    # ---- p phase: per-head softmax probabilities + transposes -------------
    p_i0 = [const.tile([128, 128], F32, name=f"p_i0_{i}") for i in range(HL)]
    p_i1 = [const.tile([128, S], F32, name=f"p_i1_{i}") for i in range(HL)]
    p16_i0 = [const.tile([128, 128], F16, name=f"p16_i0_{i}") for i in range(HL)]
    p16_i1 = [const.tile([128, S], F16, name=f"p16_i1_{i}") for i in range(HL)]
    pT_j0 = [const.tile([128, S], F32R, name=f"pT_j0_{i}") for i in range(HL)]
    pT_j1 = [const.tile([128, 128], F32R, name=f"pT_j1_{i}") for i in range(HL)]

    with tc.tile_pool(name="pp2", bufs=2, space="PSUM") as pp2, tc.tile_pool(
        name="tp2", bufs=2, space="PSUM"
    ) as tp2, tc.tile_pool(name="praw_pool", bufs=2) as praw_pool:
        for h in range(HL):
            hr = slice(h * 64, h * 64 + 64)
            for ib in range(2):
                irows = slice(ib * 128, ib * 128 + 128)
                jn = 128 if ib == 0 else 256
                m8 = m8_i0 if ib == 0 else m8_i1
                ps = pp2.tile([128, S], F32)
                nc.tensor.matmul(
                    ps[:, :jn], q_s[hr, irows], k_s[hr, 0:jn],
                    start=True, stop=False,
                )
                nc.tensor.matmul(
                    ps[:, :jn], ones_row[:, irows], al8_h[h][:, 0:jn],
                    start=False, stop=False,
                )
                nc.tensor.matmul(
                    ps[:, :jn], i128r_sb, m8[:, 0:jn], start=False, stop=True
                )
                mx = small.tile([128, 1], F32)
                nc.vector.reduce_max(mx, ps[:, :jn], axis=AX.X)
                nmx = small.tile([128, 1], F32)
                nc.vector.tensor_scalar_mul(nmx, mx, -8.0)
                ze = small.tile([128, 1], F32)
                praw = praw_pool.tile([128, S], F32)
                nc.scalar.activation(
                    praw[:, :jn], ps[:, :jn], ACT.Exp, bias=nmx, scale=8.0,
                    accum_out=ze,
                )
                rz = small.tile([128, 1], F32)
                nc.vector.reciprocal(rz, ze)
                ptile = p_i0[h] if ib == 0 else p_i1[h]
                nc.vector.tensor_scalar_mul(ptile, praw[:, :jn], rz)
                p16t = p16_i0[h] if ib == 0 else p16_i1[h]
                nc.vector.tensor_scalar_mul(p16t, praw[:, :jn], rz)
            # transposes for the context matmul
            t1 = tp2.tile([128, 128], F32)
            nc.tensor.transpose(t1, p_i0[h], i128f_sb)
            nc.scalar.copy(pT_j0[h][:, 0:128], t1)
            t2 = tp2.tile([128, 128], F32)
            nc.tensor.transpose(t2, p_i1[h][:, 0:128], i128f_sb)
            nc.scalar.copy(pT_j0[h][:, 128:256], t2)
            t3 = tp2.tile([128, 128], F32)
            nc.tensor.transpose(t3, p_i1[h][:, 128:256], i128f_sb)
            nc.scalar.copy(pT_j1[h], t3)

    # ---- context + dense partial (independent of the d loop; runs early
    # so output DMAs overlap the d loop) --------------------------------
    with tc.tile_pool(name="ctx_ps", bufs=1, space="PSUM") as ctxp_pool, tc.tile_pool(
        name="dense_ps", bufs=2, space="PSUM"
    ) as dp_pool:
        ctx_h = []
        for h in range(HL):
            rows = slice(h * 64, (h + 1) * 64)
            ctxp = ctxp_pool.tile([64, S], F32, name=f"ctxp{h}")
            nc.tensor.matmul(
                ctxp[:, 0:128], v_im[0][:, rows], pT_j0[h][:, 0:128],
                start=True, stop=True,
            )
            nc.tensor.matmul(
                ctxp[:, 128:256], v_im[0][:, rows], pT_j0[h][:, 128:256],
                start=True, stop=False,
            )
            nc.tensor.matmul(
                ctxp[:, 128:256], v_im[1][:, rows], pT_j1[h],
                start=False, stop=True,
            )
            c_sb = const.tile([64, S], F32R, name=f"ctx_sb{h}")
            nc.scalar.copy(c_sb, ctxp)
            ctx_h.append(c_sb)

        # dense partial: out[i, e] = sum_h sum_d ctxT_h[d, i] Wd_h[d, e]
        for ib in range(2):
            for eh in range(2):
                dp = dp_pool.tile([128, 512], F32)
                nc.tensor.matmul(
                    dp,
                    ctx_h[0][:, ib * 128 : (ib + 1) * 128],
                    wd_h[0][:, eh * 512 : (eh + 1) * 512],
                    start=True, stop=False,
                )
                nc.tensor.matmul(
                    dp,
                    ctx_h[1][:, ib * 128 : (ib + 1) * 128],
                    wd_h[1][:, eh * 512 : (eh + 1) * 512],
                    start=False, stop=True,
                )
                o_sb = const.tile([128, 512], F32, name=f"osb{ib}{eh}")
                if (ib + eh) % 2 == 0:
                    nc.scalar.copy(o_sb, dp)
                else:
                    nc.vector.tensor_copy(o_sb, dp)
                nc.sync.dma_start(
                    outp_d[ib * 128 : (ib + 1) * 128, eh * 512 : (eh + 1) * 512],
                    o_sb,
                )

    # ---- d loop: importance sums ------------------------------------------
    # Zb/RZ/Sb indexed [h][iblock]; columns = local d
    Zb = [[const.tile([128, D], F32, name=f"Zb{i}{j}") for j in range(2)] for i in range(HL)]
    RZ = [[const.tile([128, D], F32, name=f"RZ{i}{j}") for j in range(2)] for i in range(HL)]
    Sb = [[const.tile([128, D], F32, name=f"Sb{i}{j}") for j in range(2)] for i in range(HL)]

    with tc.tile_pool(name="x1_ps", bufs=3, space="PSUM") as x1p, tc.tile_pool(
        name="x0_ps", bufs=3, space="PSUM"
    ) as x0p, tc.tile_pool(name="tbuf1", bufs=4) as tb1, tc.tile_pool(
        name="tbuf0", bufs=4
    ) as tb0, tc.tile_pool(name="scr1", bufs=4) as scr1p, tc.tile_pool(
        name="scr0", bufs=4
    ) as scr0p:
        for h in range(HL):
            hr = slice(h * 64, h * 64 + 64)
            for t in range(D // 2):
                d0 = 2 * t
                # --- PE: build x tiles in PSUM (masked exponents)
                x1 = x1p.tile([128, 512], F32)  # i1 rows, [d0 | d1] x j(256)
                nc.tensor.matmul(
                    x1, q_s[hr, 128:256], _dup2(k_s[hr, :]),
                    start=True, stop=False,
                )
                nc.tensor.matmul(
                    x1, qneg[hr, 128:256], kdiag[hr, d0 : d0 + 2, :],
                    start=False, stop=False,
                )
                nc.tensor.matmul(
                    x1, i128r_sb, _dup2(m8_i1), start=False, stop=True
                )
                x0 = x0p.tile([128, 256], F32)  # i0 rows, [d0 | d1] x j(128)
                nc.tensor.matmul(
                    x0, q_s[hr, 0:128], _dup2(k_s[hr, 0:128]),
                    start=True, stop=False,
                )
                nc.tensor.matmul(
                    x0, qneg[hr, 0:128], kdiag[hr, d0 : d0 + 2, 0:128],
                    start=False, stop=False,
                )
                nc.tensor.matmul(
                    x0, i128r_sb, _dup2(m8_i0[:, 0:128]),
                    start=False, stop=True,
                )
                # --- ACT exps.  T tiles are fp16 so the DVE Z-sums hit the
                # 4x tensor_scalar mode.  Every third pair instead uses per-d
                # exps with the fused ACT accumulator, offloading the Z work
                # from DVE to balance the two engines.
                act_route = (t % 4 == 0)
                if act_route:
                    T1p = tb1.tile([128, 2, 256], F16)
                    T0p = tb0.tile([128, 2, 128], F16)
                    for j in range(2):
                        dl = d0 + j
                        nc.scalar.activation(
                            T1p[:, j, :], x1[:, j * 256 : (j + 1) * 256],
                            ACT.Exp, accum_out=Zb[h][1][:, dl : dl + 1],
                        )
                        nc.scalar.activation(
                            T0p[:, j, :], x0[:, j * 128 : (j + 1) * 128],
                            ACT.Exp, accum_out=Zb[h][0][:, dl : dl + 1],
                        )
                else:
                    T1p = tb1.tile([128, 2, 256], F16)
                    nc.scalar.activation(T1p.rearrange("p a b -> p (a b)"), x1,
                                         ACT.Exp)
                    T0p = tb0.tile([128, 2, 128], F16)
                    nc.scalar.activation(T0p.rearrange("p a b -> p (a b)"), x0,
                                         ACT.Exp)
                tt1 = [T1p[:, 0, :], T1p[:, 1, :]]
                tt0 = [T0p[:, 0, :], T0p[:, 1, :]]
                if not act_route:
                    for j in range(2):
                        dl = d0 + j
                        zsc1 = scr1p.tile([128, 256], F16, name="zsc1")
                        nc.vector.tensor_scalar(
                            zsc1, tt1[j], 1.0, None, mybir.AluOpType.mult,
                            mybir.AluOpType.add,
                            accum_out=Zb[h][1][:, dl : dl + 1],
                        )
                        zsc = scr0p.tile([128, 128], F16, name="zsc")
                        nc.vector.tensor_scalar(
                            zsc, tt0[j], 1.0, None, mybir.AluOpType.mult,
                            mybir.AluOpType.add,
                            accum_out=Zb[h][0][:, dl : dl + 1],
                        )
                # --- DVE: fused (T - Z p)^2 reduction (recip-free)
                for j in range(2):
                    dl = d0 + j
                    sc1 = scr1p.tile([128, 256], F16)
                    sc0 = scr0p.tile([128, 128], F16)
                    nc.vector._custom_dve(
                        SQDIFF_OP, out=sc1, in0=tt1[j], in1=p16_i1[h],
                        s0=Zb[h][1][:, dl : dl + 1],
                        accum_out=Sb[h][1][:, dl : dl + 1],
                    )
                    nc.vector._custom_dve(
                        SQDIFF_OP, out=sc0, in0=tt0[j], in1=p16_i0[h],
                        s0=Zb[h][0][:, dl : dl + 1],
                        accum_out=Sb[h][0][:, dl : dl + 1],
                    )

    # ---- importance reduction over queries (i) on PE ----------------------
    with tc.tile_pool(name="imp_ps", bufs=1, space="PSUM") as impp_pool:
        impp = impp_pool.tile([64, 2], F32)
        for h in range(HL):
            for ib in range(2):
                nc.vector.reciprocal(RZ[h][ib], Zb[h][ib])
                nc.vector.tensor_tensor(
                    RZ[h][ib], RZ[h][ib], RZ[h][ib], mybir.AluOpType.mult
                )
                nc.vector.tensor_tensor(
                    Sb[h][ib], Sb[h][ib], RZ[h][ib], mybir.AluOpType.mult
                )
            nc.tensor.matmul(
                impp[:, h : h + 1], Sb[h][0], ones_col,
                start=True, stop=False,
            )
            nc.tensor.matmul(
                impp[:, h : h + 1], Sb[h][1], ones_col,
                start=False, stop=True,
            )
        imp_sb = const.tile([64, 2], F32)
        nc.vector.tensor_copy(imp_sb, impp)
        nc.sync.dma_start(imp_d.rearrange("(h d) -> d h", d=D), imp_sb)


# ---------------------------------------------------------------------------
# Host side: build module, shard, run, unshard
# ---------------------------------------------------------------------------
_CACHE = {}


def _build_nc():
    if "nc" in _CACHE:
        return _CACHE["nc"]
    nc = bacc.Bacc(
        "TRN2",
        target_bir_lowering=False,
        debug=False,
        enable_asserts=False,
        num_devices=NC_,
    )
    ins = [
        nc.dram_tensor("hid", [KPAD, S], F32, kind="ExternalInput").ap(),
        nc.dram_tensor("what", [KPAD, 3 * 128], F32, kind="ExternalInput").ap(),
        nc.dram_tensor("wd", [128, HID], F32, kind="ExternalInput").ap(),
        nc.dram_tensor("mask8", [S, S], F32, kind="ExternalInput").ap(),
        nc.dram_tensor("al8", [HL, S], F32, kind="ExternalInput").ap(),
        nc.dram_tensor("i128", [128, 128], F32, kind="ExternalInput").ap(),
        nc.dram_tensor("ones", [1, S], F32, kind="ExternalInput").ap(),
        nc.dram_tensor("zrow", [1, D * S], F16, kind="ExternalInput").ap(),
    ]
    outs = [
        nc.dram_tensor("outp", [S, HID], F32, kind="ExternalOutput").ap(),
        nc.dram_tensor("imp", [HL * D], F32, kind="ExternalOutput").ap(),
    ]
    with tile.TileContext(nc) as tc:
        _tile_kernel(tc, outs, ins)
    nc.compile()
    _CACHE["nc"] = nc
    return nc


def _host_prep(hidden_states, residual, alibi, attention_mask, W_qkv, b_qkv,
               W_dense, b_dense):
    """Build the 8 per-core input maps (numpy only)."""
    hidden = np.asarray(hidden_states, np.float32)[0]          # [S, HID]
    mask = np.asarray(attention_mask, np.float32)[0, 0]        # [S, S]
    alibi = np.asarray(alibi, np.float32)[:, 0, :]             # [H, S]
    W_qkv = np.asarray(W_qkv, np.float32)
    b_qkv = np.asarray(b_qkv, np.float32)
    W_dense = np.asarray(W_dense, np.float32)

    hid_T1 = np.zeros((KPAD, S), np.float32)
    hid_T1[:HID] = hidden.T
    hid_T1[HID] = 1.0

    mask8 = np.ascontiguousarray(mask * 0.125)
    i128 = np.eye(128, dtype=np.float32)

    in_maps = []
    for c in range(NC_):
        heads = [2 * c, 2 * c + 1]
        qcols = np.concatenate([np.arange(h * 192, h * 192 + 64) for h in heads])
        kcols = qcols + 64
        vcols = qcols + 128
        what = np.zeros((KPAD, 3 * 128), np.float32)
        what[:HID, 0:128] = W_qkv[:, qcols] * 0.125
        what[HID, 0:128] = b_qkv[qcols] * 0.125
        what[:HID, 128:256] = W_qkv[:, kcols] * 0.125
        what[HID, 128:256] = b_qkv[kcols] * 0.125
        what[:HID, 256:384] = W_qkv[:, vcols]
        what[HID, 256:384] = b_qkv[vcols]
        wd = np.ascontiguousarray(W_dense[c * 128 : (c + 1) * 128])
        al8 = np.ascontiguousarray(alibi[heads] * 0.125)
        in_maps.append(
            dict(hid=hid_T1, what=what, wd=wd, mask8=mask8, al8=al8, i128=i128,
                 ones=np.ones((1, S), np.float32),
                 zrow=np.zeros((1, D * S), np.float16))
        )
    return in_maps


def kernel(hidden_states, residual, alibi, attention_mask, W_qkv, b_qkv,
           W_dense, b_dense, _return_results=False, **run_kwargs):
    from concourse.bass_utils import run_bass_kernel_spmd

    nc = _build_nc()
    in_maps = _host_prep(hidden_states, residual, alibi, attention_mask,
                         W_qkv, b_qkv, W_dense, b_dense)
    res = run_bass_kernel_spmd(nc, in_maps, core_ids=list(range(NC_)), **run_kwargs)
    out = np.zeros((S, HID), np.float64)
    imp = np.zeros((H * D,), np.float32)
    for c in range(NC_):
        out += res.results[c]["outp"].astype(np.float64)
        imp[c * 128 : (c + 1) * 128] = res.results[c]["imp"]
    out = (
        out.astype(np.float32)
        + np.asarray(b_dense, np.float32)
        + np.asarray(residual, np.float32)[0]
    )
    ret = (out[None], imp)
    if _return_results:
        return ret, res
    return ret


# revision 24
# speedup vs baseline: 2177.1259x; 2177.1259x over previous
"""Bloom self-attention wrapper kernel for Trainium2 (8 NeuronCores).

Shards the 16 attention heads across 8 cores (2 heads/core, tensor
parallel).  Each core computes its heads' q/k/v projection, attention
probabilities p, the per-dim softmax deviations ("qk importance"), the
attention context and a partial dense output.  The host sums the dense
partials (the all-reduce), adds bias + residual, and concatenates the
per-head importance slices.

Key device-side structure per (head, d):  the exponent tile
x_d[i,j] = (qk[i,j] - q[i,d]k[j,d])/64 + mask[i,j]/8 is built entirely
on the tensor engine in PSUM (full-qk matmul float32r + rank-1 matmul
against a block-diagonal fp16 k expansion + mask add via identity
matmul), the scalar engine computes T = exp(x) (fp16 out), Z = sum_j T
comes from either a fused ACT accumulator or a 4x-mode DVE
tensor_scalar (split between the engines to balance load), and a
custom fused DVE op accumulates sum_j (T - Z p)^2 in one pass; the
1/Z^2 scaling and the reduction over queries (a tensor-engine
ones-matmul) happen once per head at the end.  Causality halves the
work for query rows < 128 (their upper j-half is fully masked).
"""

import math
import operator
import os
import sys
from contextlib import ExitStack

import numpy as np

sys.path.insert(0, "/opt/trn_rl_repo")

import concourse.bacc as bacc  # noqa: E402
import concourse.bass as bass  # noqa: E402
import concourse.tile as tile  # noqa: E402
from concourse import mybir  # noqa: E402
from concourse._compat import with_exitstack  # noqa: E402

F32 = mybir.dt.float32
F32R = mybir.dt.float32r
F16 = mybir.dt.float16
AX = mybir.AxisListType
ACT = mybir.ActivationFunctionType

B, S, HID = 1, 256, 1024
H, D = 16, 64
NC_ = 8          # cores
HL = 2           # heads per core
KCH = 9          # contraction chunks for the (1152 = 9*128)-row padded projection
KPAD = KCH * 128


# ---------------------------------------------------------------------------
# Custom DVE op: out = (in0 - s0*in1)^2 ; accum_out = sum(out) per partition.
# With in0 = T, s0 = Z, in1 = p this gives sum_j (T - Z p)^2 = Z^2 sum (pd-p)^2
# so the reciprocal of Z moves out of the inner loop entirely.
# ---------------------------------------------------------------------------
def _register_sqdiff_op():
    from concourse import dve_ops
    from concourse.dve_spec import C0, Spec, Src0, Src1, Zero, lower, sq

    name = "SQDIFFZ_REDUCE_K"
    for op in dve_ops.OPS:
        if op.name == name:
            return op

    def _ref(in0, in1, c0, c1, c2):
        b = np.square(in0.astype(np.float32) - c0 * in1.astype(np.float32))
        b = b.astype(np.float32)
        return b, b.reshape(b.shape[0], -1).sum(axis=-1, keepdims=True)

    spec = Spec(
        body=sq(Src0 - C0 * Src1),
        accum=operator.add,
        accum_init=Zero,
        reference=_ref,
    )
    row = dve_ops._CUSTOM_DVE_ROW_BASE + len(dve_ops.OPS)
    assert row < 0x20, "custom DVE opcode row overflow"
    from concourse.dve_uop import DveOpSpec

    shas = {}
    for ver in ("v3", "v4"):
        try:
            tmp = DveOpSpec(
                name=name, opcode=row, uops=lower(spec, ver=ver), rd1_en=True
            )
            shas[ver] = tmp.sha(ver)
        except Exception:
            pass
    op = dve_ops.DveOp(name, spec, subdim=False, uops_sha=shas)
    dve_ops.OPS.append(op)
    dve_ops._SUB_OPCODE_FOR_NAME[name] = row
    dve_ops.CUSTOM_DVE_SPECS[name] = spec
    return op


SQDIFF_OP = _register_sqdiff_op()
NO_CUSTOM = bool(int(os.environ.get("KERNEL_NO_CUSTOM", "0")))
TINY = bool(int(os.environ.get("KERNEL_TINY", "0")))


def _dup2(ap):
    """[P, N] access pattern -> [P, 2, N] with a step-0 middle dim."""
    assert len(ap.ap) == 2, ap.ap
    return bass.AP(
        tensor=ap.tensor, offset=ap.offset, ap=[ap.ap[0], [0, 2], ap.ap[1]]
    )


def _r(ap):
    return ap.bitcast(F32R)


# ---------------------------------------------------------------------------
# Device kernel (one core, SPMD: identical program, per-core data)
# ---------------------------------------------------------------------------
@with_exitstack
def _tile_kernel(ctx: ExitStack, tc: tile.TileContext, outs, ins):
    nc = tc.nc
    hid_d, what_d, wd_d, mask8_d, al8_d, i128_d, ones_d, zrow_d = ins
    outp_d, imp_d = outs

    const = ctx.enter_context(tc.tile_pool(name="const", bufs=1))
    small = ctx.enter_context(tc.tile_pool(name="small", bufs=4))

    # ---- input DMA ---------------------------------------------------------
    # Tiles feeding float32r matmuls are declared float32r (BIR verifier
    # requires f32r-matmul operands to be produced as f32r).
    hid_sb = const.tile([128, KCH, S], F32R)
    what_sb = const.tile([128, KCH, 3 * 128], F32R)
    hid_r = hid_d.rearrange("(c p) n -> p c n", p=128).bitcast(F32R)
    what_r = what_d.rearrange("(c p) n -> p c n", p=128).bitcast(F32R)
    for kc in range(KCH):
        nc.gpsimd.dma_start(what_sb[:, kc, :], what_r[:, kc, :])
        nc.gpsimd.dma_start(hid_sb[:, kc, :], hid_r[:, kc, :])
    wd_h = []
    for hh in range(HL):
        w = const.tile([64, 1024], F32R, name=f"wd_h{hh}")
        nc.sync.dma_start(w, wd_d[hh * 64 : (hh + 1) * 64].bitcast(F32R))
        wd_h.append(w)
    m8_i0 = const.tile([128, S], F32R)
    nc.sync.dma_start(m8_i0, mask8_d[0:128].bitcast(F32R))
    m8_i1 = const.tile([128, S], F32R)
    nc.sync.dma_start(m8_i1, mask8_d[128:256].bitcast(F32R))
    i128r_sb = const.tile([128, 128], F32R)
    nc.sync.dma_start(i128r_sb, i128_d.bitcast(F32R))
    i128f_sb = const.tile([128, 128], F32)
    nc.sync.dma_start(i128f_sb, i128_d)
    al8_h = []
    for hh in range(HL):
        a = const.tile([1, S], F32R, name=f"al8_h{hh}")
        nc.sync.dma_start(a, al8_d[hh : hh + 1].bitcast(F32R))
        al8_h.append(a)

    ones_row = const.tile([1, S], F32R)
    nc.sync.dma_start(ones_row, ones_d.bitcast(F32R))

    dramp = ctx.enter_context(tc.tile_pool(name="dramp", bufs=1, space="DRAM"))
    kdimg = dramp.tile([128, D * S], F16)
    NCH = 4
    CW = D // NCH
    for ch in range(NCH):
        zbc = bass.AP(tensor=zrow_d.tensor, offset=zrow_d.offset,
                      ap=[[0, 128], [1, CW * S]])
        nc.sync.dma_start(kdimg[:, ch * CW * S : (ch + 1) * CW * S], zbc)
    ones_col = const.tile([128, 1], F32)
    nc.vector.memset(ones_col, 1.0)

    # kdiag: row r = (h d'), free [d, j]; kdiag[r, d, :] = k_s[r, :] iff
    # d == r mod 64 else 0.  Lets the per-d rank-1 subtraction run as an
    # aligned K=64 matmul (PE requires quadrant-aligned partition bases).
    # Allocated + zeroed up front so its lifetime overlaps every const tile.
    kdiag = const.tile([128, D, S], F16)

    # ---- projection: q_s, k_s (scaled 1/8, bias folded), v ----------------
    q_s = const.tile([128, S], F32R)     # [(h d), i]
    k_s = const.tile([128, S], F32R)     # [(h d), j]
    k_f16 = const.tile([128, S], F16)
    qneg = const.tile([128, S], F16)
    v_dm = const.tile([128, S], F32)    # [(h d), j]
    v_im = [const.tile([128, 128], F32R, name=f"v_im{i}") for i in range(2)]  # [j, (h d)] blocks

    with tc.tile_pool(name="proj_ps", bufs=2, space="PSUM") as pp, tc.tile_pool(
        name="tr_ps", bufs=2, space="PSUM"
    ) as tp:
        for idx, dst in enumerate((q_s, k_s, v_dm)):
            ps = pp.tile([128, S], F32)
            cb = idx * 128
            for kc in range(KCH):
                nc.tensor.matmul(
                    ps,
                    what_sb[:, kc, cb : cb + 128],
                    hid_sb[:, kc, :],
                    start=(kc == 0),
                    stop=(kc == KCH - 1),
                )
            nc.scalar.copy(dst, ps)
            if idx == 1:
                nc.scalar.copy(k_f16, ps)
        for ib in range(2):
            tps = tp.tile([128, 128], F32)
            nc.tensor.transpose(tps, v_dm[:, ib * 128 : (ib + 1) * 128], i128f_sb)
            nc.scalar.copy(v_im[ib], tps)
    nc.scalar.mul(qneg, q_s.bitcast(F32), -1.0)

    # kdiag built via a DRAM image, pipelined in NCH chunks of CW diagonal
    # positions: zero-fill (issued up top), strided scatter (the diagonal
    # destination offset r*16384 + (r%64)*256 is linear in r within each
    # 64-row half), then readback into SBUF.
    for ch in range(NCH):
        r0 = ch * CW
        for half in range(2):
            dst = bass.AP(
                tensor=kdimg.tensor,
                offset=kdimg.offset + half * 64 * (D * S) + r0 * (D * S + S),
                ap=[[D * S + S, CW], [1, S]],
            )
            nc.sync.dma_start(
                dst, k_f16[half * 64 + r0 : half * 64 + r0 + CW, :]
            )
        nc.sync.dma_start(
            kdiag[:, r0 : r0 + CW, :],
            kdimg[:, r0 * S : (r0 + CW) * S],
        )

    # ---- p phase: per-head softmax probabilities + transposes -------------
    p_i0 = [const.tile([128, 128], F32, name=f"p_i0_{i}") for i in range(HL)]
    p_i1 = [const.tile([128, S], F32, name=f"p_i1_{i}") for i in range(HL)]
    p16_i0 = [const.tile([128, 128], F16, name=f"p16_i0_{i}") for i in range(HL)]
    p16_i1 = [const.tile([128, S], F16, name=f"p16_i1_{i}") for i in range(HL)]
    pT_j0 = [const.tile([128, S], F32R, name=f"pT_j0_{i}") for i in range(HL)]
    pT_j1 = [const.tile([128, 128], F32R, name=f"pT_j1_{i}") for i in range(HL)]

    with tc.tile_pool(name="pp2", bufs=2, space="PSUM") as pp2, tc.tile_pool(
        name="tp2", bufs=2, space="PSUM"
    ) as tp2, tc.tile_pool(name="praw_pool", bufs=2) as praw_pool:
        for h in range(HL):
            hr = slice(h * 64, h * 64 + 64)
            for ib in range(2):
                irows = slice(ib * 128, ib * 128 + 128)
                jn = 128 if ib == 0 else 256
                m8 = m8_i0 if ib == 0 else m8_i1
                ps = pp2.tile([128, S], F32)
                nc.tensor.matmul(
                    ps[:, :jn], q_s[hr, irows], k_s[hr, 0:jn],
                    start=True, stop=False,
                )
                nc.tensor.matmul(
                    ps[:, :jn], ones_row[:, irows], al8_h[h][:, 0:jn],
                    start=False, stop=False,
                )
                nc.tensor.matmul(
                    ps[:, :jn], i128r_sb, m8[:, 0:jn], start=False, stop=True
                )
                mx = small.tile([128, 1], F32)
                nc.vector.reduce_max(mx, ps[:, :jn], axis=AX.X)
                nmx = small.tile([128, 1], F32)
                nc.vector.tensor_scalar_mul(nmx, mx, -8.0)
                ze = small.tile([128, 1], F32)
                praw = praw_pool.tile([128, S], F32)
                nc.scalar.activation(
                    praw[:, :jn], ps[:, :jn], ACT.Exp, bias=nmx, scale=8.0,
                    accum_out=ze,
                )
                rz = small.tile([128, 1], F32)
                nc.vector.reciprocal(rz, ze)
                ptile = p_i0[h] if ib == 0 else p_i1[h]
                nc.vector.tensor_scalar_mul(ptile, praw[:, :jn], rz)
                p16t = p16_i0[h] if ib == 0 else p16_i1[h]
                nc.vector.tensor_scalar_mul(p16t, praw[:, :jn], rz)
            # transposes for the context matmul
            t1 = tp2.tile([128, 128], F32)
            nc.tensor.transpose(t1, p_i0[h], i128f_sb)
            nc.scalar.copy(pT_j0[h][:, 0:128], t1)
            t2 = tp2.tile([128, 128], F32)
            nc.tensor.transpose(t2, p_i1[h][:, 0:128], i128f_sb)
            nc.scalar.copy(pT_j0[h][:, 128:256], t2)
            t3 = tp2.tile([128, 128], F32)
            nc.tensor.transpose(t3, p_i1[h][:, 128:256], i128f_sb)
            nc.scalar.copy(pT_j1[h], t3)

    # ---- context + dense partial (independent of the d loop; runs early
    # so output DMAs overlap the d loop) --------------------------------
    with tc.tile_pool(name="ctx_ps", bufs=1, space="PSUM") as ctxp_pool, tc.tile_pool(
        name="dense_ps", bufs=2, space="PSUM"
    ) as dp_pool:
        ctx_h = []
        for h in range(HL):
            rows = slice(h * 64, (h + 1) * 64)
            ctxp = ctxp_pool.tile([64, S], F32, name=f"ctxp{h}")
            nc.tensor.matmul(
                ctxp[:, 0:128], v_im[0][:, rows], pT_j0[h][:, 0:128],
                start=True, stop=True,
            )
            nc.tensor.matmul(
                ctxp[:, 128:256], v_im[0][:, rows], pT_j0[h][:, 128:256],
                start=True, stop=False,
            )
            nc.tensor.matmul(
                ctxp[:, 128:256], v_im[1][:, rows], pT_j1[h],
                start=False, stop=True,
            )
            c_sb = const.tile([64, S], F32R, name=f"ctx_sb{h}")
            nc.scalar.copy(c_sb, ctxp)
            ctx_h.append(c_sb)

        # dense partial: out[i, e] = sum_h sum_d ctxT_h[d, i] Wd_h[d, e]
        for ib in range(2):
            for eh in range(2):
                dp = dp_pool.tile([128, 512], F32)
                nc.tensor.matmul(
                    dp,
                    ctx_h[0][:, ib * 128 : (ib + 1) * 128],
                    wd_h[0][:, eh * 512 : (eh + 1) * 512],
                    start=True, stop=False,
                )
                nc.tensor.matmul(
                    dp,
                    ctx_h[1][:, ib * 128 : (ib + 1) * 128],
                    wd_h[1][:, eh * 512 : (eh + 1) * 512],
                    start=False, stop=True,
                )
                o_sb = const.tile([128, 512], F32, name=f"osb{ib}{eh}")
                if (ib + eh) % 2 == 0:
                    nc.scalar.copy(o_sb, dp)
                else:
                    nc.vector.tensor_copy(o_sb, dp)
                nc.sync.dma_start(
                    outp_d[ib * 128 : (ib + 1) * 128, eh * 512 : (eh + 1) * 512],
                    o_sb,
                )

    # ---- d loop: importance sums ------------------------------------------
    # Zb/RZ/Sb indexed [h][iblock]; columns = local d
    Zb = [[const.tile([128, D], F32, name=f"Zb{i}{j}") for j in range(2)] for i in range(HL)]
    RZ = [[const.tile([128, D], F32, name=f"RZ{i}{j}") for j in range(2)] for i in range(HL)]
    Sb = [[const.tile([128, D], F32, name=f"Sb{i}{j}") for j in range(2)] for i in range(HL)]

    with tc.tile_pool(name="x1_ps", bufs=3, space="PSUM") as x1p, tc.tile_pool(
        name="x0_ps", bufs=3, space="PSUM"
    ) as x0p, tc.tile_pool(name="tbuf1", bufs=4) as tb1, tc.tile_pool(
        name="tbuf0", bufs=4
    ) as tb0, tc.tile_pool(name="scr1", bufs=4) as scr1p, tc.tile_pool(
        name="scr0", bufs=4
    ) as scr0p:
        for h in range(HL):
            hr = slice(h * 64, h * 64 + 64)
            for t in range(2 if TINY else D // 2):
                d0 = 2 * t
                # --- PE: build x tiles in PSUM (masked exponents)
                x1 = x1p.tile([128, 512], F32)  # i1 rows, [d0 | d1] x j(256)
                nc.tensor.matmul(
                    x1, q_s[hr, 128:256], _dup2(k_s[hr, :]),
                    start=True, stop=False,
                )
                nc.tensor.matmul(
                    x1, qneg[hr, 128:256], kdiag[hr, d0 : d0 + 2, :],
                    start=False, stop=False,
                )
                nc.tensor.matmul(
                    x1, i128r_sb, _dup2(m8_i1), start=False, stop=True
                )
                x0 = x0p.tile([128, 256], F32)  # i0 rows, [d0 | d1] x j(128)
                nc.tensor.matmul(
                    x0, q_s[hr, 0:128], _dup2(k_s[hr, 0:128]),
                    start=True, stop=False,
                )
                nc.tensor.matmul(
                    x0, qneg[hr, 0:128], kdiag[hr, d0 : d0 + 2, 0:128],
                    start=False, stop=False,
                )
                nc.tensor.matmul(
                    x0, i128r_sb, _dup2(m8_i0[:, 0:128]),
                    start=False, stop=True,
                )
                # --- ACT exps.  T tiles are fp16 so the DVE Z-sums hit the
                # 4x tensor_scalar mode.  Every third pair instead uses per-d
                # exps with the fused ACT accumulator, offloading the Z work
                # from DVE to balance the two engines.
                act_route = (t % 2 == 0)
                if act_route:
                    T1p = tb1.tile([128, 2, 256], F16)
                    T0p = tb0.tile([128, 2, 128], F16)
                    for j in range(2):
                        dl = d0 + j
                        nc.scalar.activation(
                            T1p[:, j, :], x1[:, j * 256 : (j + 1) * 256],
                            ACT.Exp, accum_out=Zb[h][1][:, dl : dl + 1],
                        )
                        nc.scalar.activation(
                            T0p[:, j, :], x0[:, j * 128 : (j + 1) * 128],
                            ACT.Exp, accum_out=Zb[h][0][:, dl : dl + 1],
                        )
                else:
                    T1p = tb1.tile([128, 2, 256], F16)
                    nc.scalar.activation(T1p.rearrange("p a b -> p (a b)"), x1,
                                         ACT.Exp)
                    T0p = tb0.tile([128, 2, 128], F16)
                    nc.scalar.activation(T0p.rearrange("p a b -> p (a b)"), x0,
                                         ACT.Exp)
                tt1 = [T1p[:, 0, :], T1p[:, 1, :]]
                tt0 = [T0p[:, 0, :], T0p[:, 1, :]]
                if not act_route:
                    for j in range(2):
                        dl = d0 + j
                        zsc1 = scr1p.tile([128, 256], F16, name="zsc1")
                        nc.vector.tensor_scalar(
                            zsc1, tt1[j], 1.0, None, mybir.AluOpType.mult,
                            mybir.AluOpType.add,
                            accum_out=Zb[h][1][:, dl : dl + 1],
                        )
                        zsc = scr0p.tile([128, 128], F16, name="zsc")
                        nc.vector.tensor_scalar(
                            zsc, tt0[j], 1.0, None, mybir.AluOpType.mult,
                            mybir.AluOpType.add,
                            accum_out=Zb[h][0][:, dl : dl + 1],
                        )
                # --- DVE: fused (T - Z p)^2 reduction (recip-free)
                for j in range(2):
                    dl = d0 + j
                    sc1 = scr1p.tile([128, 256], F16)
                    sc0 = scr0p.tile([128, 128], F16)
                    nc.vector._custom_dve(
                        SQDIFF_OP, out=sc1, in0=tt1[j], in1=p16_i1[h],
                        s0=Zb[h][1][:, dl : dl + 1],
                        accum_out=Sb[h][1][:, dl : dl + 1],
                    )
                    nc.vector._custom_dve(
                        SQDIFF_OP, out=sc0, in0=tt0[j], in1=p16_i0[h],
                        s0=Zb[h][0][:, dl : dl + 1],
                        accum_out=Sb[h][0][:, dl : dl + 1],
                    )

    # ---- importance reduction over queries (i) on PE ----------------------
    with tc.tile_pool(name="imp_ps", bufs=1, space="PSUM") as impp_pool:
        impp = impp_pool.tile([64, 2], F32)
        for h in range(HL):
            for ib in range(2):
                nc.vector.reciprocal(RZ[h][ib], Zb[h][ib])
                nc.vector.tensor_tensor(
                    RZ[h][ib], RZ[h][ib], RZ[h][ib], mybir.AluOpType.mult
                )
                nc.vector.tensor_tensor(
                    Sb[h][ib], Sb[h][ib], RZ[h][ib], mybir.AluOpType.mult
                )
            nc.tensor.matmul(
                impp[:, h : h + 1], Sb[h][0], ones_col,
                start=True, stop=False,
            )
            nc.tensor.matmul(
                impp[:, h : h + 1], Sb[h][1], ones_col,
                start=False, stop=True,
            )
        imp_sb = const.tile([64, 2], F32)
        nc.vector.tensor_copy(imp_sb, impp)
        nc.sync.dma_start(imp_d.rearrange("(h d) -> d h", d=D), imp_sb)


# ---------------------------------------------------------------------------
# Host side: build module, shard, run, unshard
# ---------------------------------------------------------------------------
_CACHE = {}


def _build_nc():
    if "nc" in _CACHE:
        return _CACHE["nc"]
    nc = bacc.Bacc(
        "TRN2",
        target_bir_lowering=False,
        debug=False,
        enable_asserts=False,
        num_devices=NC_,
    )
    ins = [
        nc.dram_tensor("hid", [KPAD, S], F32, kind="ExternalInput").ap(),
        nc.dram_tensor("what", [KPAD, 3 * 128], F32, kind="ExternalInput").ap(),
        nc.dram_tensor("wd", [128, HID], F32, kind="ExternalInput").ap(),
        nc.dram_tensor("mask8", [S, S], F32, kind="ExternalInput").ap(),
        nc.dram_tensor("al8", [HL, S], F32, kind="ExternalInput").ap(),
        nc.dram_tensor("i128", [128, 128], F32, kind="ExternalInput").ap(),
        nc.dram_tensor("ones", [1, S], F32, kind="ExternalInput").ap(),
        nc.dram_tensor("zrow", [1, D * S], F16, kind="ExternalInput").ap(),
    ]
    outs = [
        nc.dram_tensor("outp", [S, HID], F32, kind="ExternalOutput").ap(),
        nc.dram_tensor("imp", [HL * D], F32, kind="ExternalOutput").ap(),
    ]
    with tile.TileContext(nc) as tc:
        _tile_kernel(tc, outs, ins)
    nc.compile()
    _CACHE["nc"] = nc
    return nc


def _host_prep(hidden_states, residual, alibi, attention_mask, W_qkv, b_qkv,
               W_dense, b_dense):
    """Build the 8 per-core input maps (numpy only)."""
    hidden = np.asarray(hidden_states, np.float32)[0]          # [S, HID]
    mask = np.asarray(attention_mask, np.float32)[0, 0]        # [S, S]
    alibi = np.asarray(alibi, np.float32)[:, 0, :]             # [H, S]
    W_qkv = np.asarray(W_qkv, np.float32)
    b_qkv = np.asarray(b_qkv, np.float32)
    W_dense = np.asarray(W_dense, np.float32)

    hid_T1 = np.zeros((KPAD, S), np.float32)
    hid_T1[:HID] = hidden.T
    hid_T1[HID] = 1.0

    mask8 = np.ascontiguousarray(mask * 0.125)
    i128 = np.eye(128, dtype=np.float32)

    in_maps = []
    for c in range(NC_):
        heads = [2 * c, 2 * c + 1]
        qcols = np.concatenate([np.arange(h * 192, h * 192 + 64) for h in heads])
        kcols = qcols + 64
        vcols = qcols + 128
        what = np.zeros((KPAD, 3 * 128), np.float32)
        what[:HID, 0:128] = W_qkv[:, qcols] * 0.125
        what[HID, 0:128] = b_qkv[qcols] * 0.125
        what[:HID, 128:256] = W_qkv[:, kcols] * 0.125
        what[HID, 128:256] = b_qkv[kcols] * 0.125
        what[:HID, 256:384] = W_qkv[:, vcols]
        what[HID, 256:384] = b_qkv[vcols]
        wd = np.ascontiguousarray(W_dense[c * 128 : (c + 1) * 128])
        al8 = np.ascontiguousarray(alibi[heads] * 0.125)
        in_maps.append(
            dict(hid=hid_T1, what=what, wd=wd, mask8=mask8, al8=al8, i128=i128,
                 ones=np.ones((1, S), np.float32),
                 zrow=np.zeros((1, D * S), np.float16))
        )
    return in_maps


def kernel(hidden_states, residual, alibi, attention_mask, W_qkv, b_qkv,
           W_dense, b_dense, _return_results=False, **run_kwargs):
    from concourse.bass_utils import run_bass_kernel_spmd

    nc = _build_nc()
    in_maps = _host_prep(hidden_states, residual, alibi, attention_mask,
                         W_qkv, b_qkv, W_dense, b_dense)
    res = run_bass_kernel_spmd(nc, in_maps, core_ids=list(range(NC_)), **run_kwargs)
    out = np.zeros((S, HID), np.float64)
    imp = np.zeros((H * D,), np.float32)
    for c in range(NC_):
        out += res.results[c]["outp"].astype(np.float64)
        imp[c * 128 : (c + 1) * 128] = res.results[c]["imp"]
    out = (
        out.astype(np.float32)
        + np.asarray(b_dense, np.float32)
        + np.asarray(residual, np.float32)[0]
    )
    ret = (out[None], imp)
    if _return_results:
        return ret, res
    return ret
